# revision 1
# baseline (speedup 1.0000x reference)
"""Single-head causal attention with RoPE on 8 TRN2 NeuronCores.

Sharding: core c -> batch c//2, parity p = c%2 takes the interleaved
512-row q-blocks {p, p+2, p+4, p+6} of T=4096 (causal load balance).
Each core computes full K/V for its batch (duplicated across the pair),
so no collectives are needed.

Device layout tricks:
- xT passed host-transposed and column-permuted into "slot" order
  [own q-blocks | other blocks] so the SPMD program is identical on all
  cores (q projection always for t-slots 0..15).
- Wq/Wk rows host-permuted evens-first so RoPE becomes rotate-half form
  (free-dim ops only); scores are permutation-invariant.
- Scores computed transposed (S^T[s, q]) so softmax P^T feeds the AV
  matmul directly; row sums via ones-vector matmuls; causal masking via
  exp bias (-1e9) for the data-dependent tail block plus a static
  triangular multiplicative mask for the diagonal block.
"""
import numpy as np

B, T, C, HD = 4, 4096, 2048, 128
P = 128
NB = 8          # 512-row blocks per sequence
BS = 512        # block size
SCALE = float(C) ** -0.5
NEG = -1.0e9


def build():
    import concourse.bass as bass
    import concourse.mybir as mybir
    import bass_rust
    from concourse.tile import TileContext
    from concourse.masks import make_identity

    f32 = mybir.dt.float32
    f32r = mybir.dt.float32r
    EXP = mybir.ActivationFunctionType.Exp

    nc = bass.Bass()
    xt = nc.declare_dram_parameter("xt", [C, T], f32, isOutput=False)
    w = nc.declare_dram_parameter("w", [C, 3 * HD], f32, isOutput=False)
    cos2 = nc.declare_dram_parameter("cos2", [T, P], f32, isOutput=False)
    sin2 = nc.declare_dram_parameter("sin2", [T, P], f32, isOutput=False)
    tailb = nc.declare_dram_parameter("tailb", [P, 1], f32, isOutput=False)
    out = nc.declare_dram_parameter("out", [T // 2, HD], f32, isOutput=True)

    xtr = xt.bitcast(f32r)
    wr = w.bitcast(f32r)

    with TileContext(nc) as tc:
        with (
            tc.tile_pool(name="const", bufs=1) as cp,
            tc.tile_pool(name="xp", bufs=2) as xp,
            tc.tile_pool(name="rot", bufs=2) as rp,
            tc.tile_pool(name="pt", bufs=3) as ptp,
            tc.tile_pool(name="osb", bufs=2) as osb,
            tc.tile_pool(name="pps", bufs=2, space="PSUM") as pps,
            tc.tile_pool(name="tps", bufs=2, space="PSUM") as tps,
            tc.tile_pool(name="sps", bufs=2, space="PSUM") as sps,
            tc.tile_pool(name="o2ps", bufs=1, space="PSUM") as o2ps,
            tc.tile_pool(name="smps", bufs=1, space="PSUM") as smps,
        ):
            # ---- constants / resident tensors ----
            ident = cp.tile([P, P], f32, tag="ident")
            make_identity(nc, ident[:])
            ones = cp.tile([P, 2], f32, tag="ones")
            nc.gpsimd.memset(ones[:], 1.0)
            tri = cp.tile([P, 4 * BS], f32, tag="tri")
            nc.gpsimd.memset(tri[:], 0.0)
            for j in range(4):
                # tri_j[s, q] = 1.0 where s + 128*j <= q else 0.0
                nc.gpsimd.affine_select(
                    out=tri[:, j * BS:(j + 1) * BS],
                    in_=tri[:, j * BS:(j + 1) * BS],
                    compare_op=mybir.AluOpType.is_gt,
                    fill=1.0, base=j * P,
                    pattern=[[-1, BS]], channel_multiplier=1,
                )
            wt = cp.tile([P, 16 * 384], f32r, tag="wt")
            for g in range(4):   # 4 DMAs -> 4 queues
                nc.sync.dma_start(
                    wt[:, g * 4 * 384:(g + 1) * 4 * 384].rearrange(
                        "p (k n) -> p k n", k=4),
                    wr[g * 512:(g + 1) * 512, :].rearrange(
                        "(k p) n -> p k n", p=P))
            cst = cp.tile([P, 32 * P], f32, tag="cst")
            snt = cp.tile([P, 32 * P], f32, tag="snt")
            for g in range(4):
                sl = slice(g * 8 * P, (g + 1) * 8 * P)
                nc.sync.dma_start(
                    cst[:, sl].rearrange("p (k n) -> p k n", k=8),
                    cos2[g * 8 * P:(g + 1) * 8 * P, :].rearrange(
                        "(k p) n -> p k n", p=P))
                nc.sync.dma_start(
                    snt[:, sl].rearrange("p (k n) -> p k n", k=8),
                    sin2[g * 8 * P:(g + 1) * 8 * P, :].rearrange(
                        "(k p) n -> p k n", p=P))
            tb = cp.tile([P, 1], f32, tag="tb")
            nc.sync.dma_start(tb[:], tailb[:])

            qT = cp.tile([P, 16 * P], f32r, tag="qT")   # [d, 2048]
            kT = cp.tile([P, 32 * P], f32r, tag="kT")   # [d, 4096]
            vsb = cp.tile([P, 32 * P], f32r, tag="vsb")  # v[s,d] by s-tile

            # ---- phase 1: joint projection + RoPE + transposes ----
            for tg in range(8):          # t-groups of 512 (slot order)
                xts = []
                for ci in range(16):
                    xtile = xp.tile([P, BS], f32r, tag=f"x{ci}")
                    nc.sync.dma_start(
                        xtile[:], xtr[ci * P:(ci + 1) * P,
                                      tg * BS:(tg + 1) * BS])
                    xts.append(xtile)
                for sub in range(4):
                    t128 = tg * 4 + sub
                    nq = 384 if t128 < 16 else 256   # [k|v|q] layout
                    pp = pps.tile([P, 384], f32, tag="pp")
                    for ci in range(16):
                        nc.tensor.matmul(
                            pp[:, 0:nq],
                            xts[ci][:, sub * P:(sub + 1) * P],
                            wt[:, ci * 384:ci * 384 + nq],
                            start=(ci == 0), stop=(ci == 15))
                    cs = cst[:, t128 * P:(t128 + 1) * P]
                    sn = snt[:, t128 * P:(t128 + 1) * P]
                    H = 64

                    def rope(src_off, dst):
                        s0 = pp[:, src_off:src_off + P]
                        nc.vector.tensor_mul(dst[:], s0, cs)
                        tmp = rp.tile([P, P], f32, tag="ropetmp")
                        nc.vector.tensor_mul(
                            tmp[:, 0:H], pp[:, src_off + H:src_off + P],
                            sn[:, 0:H])
                        nc.vector.tensor_mul(
                            tmp[:, H:P], pp[:, src_off:src_off + H],
                            sn[:, H:P])
                        nc.vector.tensor_add(dst[:], dst[:], tmp[:])

                    rk = rp.tile([P, P], f32, tag="rk")
                    rope(0, rk)
                    nc.scalar.copy(vsb[:, t128 * P:(t128 + 1) * P],
                                   pp[:, P:2 * P])
                    tpk = tps.tile([P, P], f32, tag="tp")
                    nc.tensor.transpose(tpk[:], rk[:], ident[:])
                    nc.scalar.copy(kT[:, t128 * P:(t128 + 1) * P], tpk[:])
                    if t128 < 16:
                        rq = rp.tile([P, P], f32, tag="rq")
                        rope(2 * P, rq)
                        tpq = tps.tile([P, P], f32, tag="tp")
                        nc.tensor.transpose(tpq[:], rq[:], ident[:])
                        nc.scalar.copy(qT[:, t128 * P:(t128 + 1) * P],
                                       tpq[:])

            # ---- phase 2: attention per q-slot ----
            for j in range(4):
                qsl = slice(j * BS, (j + 1) * BS)
                o2 = o2ps.tile([P, BS], f32, tag="o2")
                sm = smps.tile([1, BS], f32, tag="sm")
                slots = ([(s, "full") for s in range(j)]
                         + [(4 + s, "full") for s in range(j)]
                         + [(j, "diag"), (4 + j, "tail")])
                nmm = len(slots) * 4
                mm = 0
                for (si, kind) in slots:
                    for st in range(4):
                        scol = si * BS + st * P
                        Sps = sps.tile([P, BS], f32, tag="S")
                        nc.tensor.matmul(Sps[:], kT[:, scol:scol + P],
                                         qT[:, qsl], start=True, stop=True)
                        Pt = ptp.tile([P, BS], f32r, tag="Pt")
                        bias = tb[:, 0:1] if kind == "tail" else 0.0
                        nc.scalar.activation(Pt[:], Sps[:], EXP,
                                             bias=bias, scale=SCALE)
                        if kind == "diag":
                            nc.vector.tensor_mul(
                                Pt[:], Pt[:], tri[:, st * BS:(st + 1) * BS])
                        nc.tensor.matmul(o2[:], vsb[:, scol:scol + P], Pt[:],
                                         start=(mm == 0), stop=(mm == nmm - 1))
                        nc.tensor.matmul(sm[:], ones[:, 0:1].bitcast(f32r), Pt[:],
                                         start=(mm == 0), stop=(mm == nmm - 1))
                        mm += 1
                # normalize + transpose + store
                smsb = osb.tile([1, BS], f32, tag="smsb")
                nc.scalar.copy(smsb[:], sm[:])
                o2sb = osb.tile([P, BS], f32, tag="o2sb")
                nc.scalar.copy(o2sb[:], o2[:])
                rcp = osb.tile([P, 4], f32, tag="rcp")
                for ch in range(4):
                    rs = tps.tile([P, 1], f32, tag="tp")
                    nc.tensor.transpose(rs[:], smsb[0:1, ch * P:(ch + 1) * P],
                                        ident[0:1, 0:1])
                    nc.vector.reciprocal(rcp[:, ch:ch + 1], rs[:])
                for ch in range(4):
                    ot = tps.tile([P, P], f32, tag="tp")
                    nc.tensor.transpose(ot[:], o2sb[:, ch * P:(ch + 1) * P],
                                        ident[:])
                    osbt = osb.tile([P, P], f32, tag="ofin")
                    nc.vector.tensor_scalar_mul(osbt[:], ot[:],
                                                rcp[:, ch:ch + 1])
                    r0 = j * BS + ch * P
                    nc.sync.dma_start(out[r0:r0 + P, :], osbt[:])

    bass_rust.generate_event_semaphores(nc)
    return nc


_CACHE = {}


def _get_nc():
    if "nc" not in _CACHE:
        _CACHE["nc"] = build()
    return _CACHE["nc"]


def _prep_inputs(x, Wq, Wk, Wv, cos, sin):
    perm = np.concatenate([np.arange(0, HD, 2), np.arange(1, HD, 2)])
    wq = Wq[perm].astype(np.float32)
    wk = Wk[perm].astype(np.float32)
    w = np.concatenate([wk.T, Wv.T.astype(np.float32), wq.T], axis=1)
    w = np.ascontiguousarray(w)  # [C, 384] = [k|v|q]
    cos2 = np.concatenate([cos, cos], axis=1).astype(np.float32)
    sin2 = np.concatenate([-sin, sin], axis=1).astype(np.float32)
    in_maps = []
    orders = []
    for c in range(8):
        b, par = c // 2, c % 2
        order = [par, par + 2, par + 4, par + 6,
                 1 - par, 3 - par, 5 - par, 7 - par]
        orders.append(order)
        xb = np.asarray(x[b], np.float32)          # [T, C]
        xtp = np.empty((C, T), np.float32)
        c2 = np.empty((T, P), np.float32)
        s2 = np.empty((T, P), np.float32)
        for sl, ab in enumerate(order):
            dst = slice(sl * BS, (sl + 1) * BS)
            src = slice(ab * BS, (ab + 1) * BS)
            xtp[:, dst] = xb[src].T
            c2[dst] = cos2[src]
            s2[dst] = sin2[src]
        tailb = np.full((P, 1), NEG if par == 0 else 0.0, np.float32)
        in_maps.append({"xt": np.ascontiguousarray(xtp), "w": w,
                        "cos2": np.ascontiguousarray(c2),
                        "sin2": np.ascontiguousarray(s2), "tailb": tailb})
    return in_maps, orders


def _run(x, Wq, Wk, Wv, cos, sin, trace=False):
    from concourse.bass_utils import run_bass_kernel_spmd
    nc = _get_nc()
    in_maps, orders = _prep_inputs(x, Wq, Wk, Wv, cos, sin)
    res = run_bass_kernel_spmd(nc, in_maps, list(range(8)), trace=trace)
    full = np.empty((B, T, HD), np.float32)
    for c in range(8):
        b, order = c // 2, orders[c]
        oc = res.results[c]["out"]
        for j in range(4):
            ab = order[j]
            full[b, ab * BS:(ab + 1) * BS] = oc[j * BS:(j + 1) * BS]
    return full, res


def kernel(x, Wq, Wk, Wv, cos, sin):
    return _run(x, Wq, Wk, Wv, cos, sin, trace=False)[0]



# revision 9
# speedup vs baseline: 1.2428x; 1.2428x over previous
"""Single-head causal attention with RoPE on 8 TRN2 NeuronCores (v2).

Sharding: core c -> batch c//2, parity p = c%2 owns the interleaved
512-row q-blocks {p, p+2, p+4, p+6} of T=4096. Each core projects
q/k/v + RoPE only for its OWN 2048 rows (halves x DMA + projection
matmuls); pairs exchange post-RoPE kT and V via chunked fp16
AllGathers (one per 512-block group) into a rank-ordered layout
(rank0 blocks = kT slots 0-3, rank1 = slots 4-7), which is
core-independent so the SPMD program is identical on all cores.

Causal structure per q-slot j: full passes on kT slots 0..j-1 and
4..4+j-1, pass A on slot j (diag for p=0 / full for p=1), pass B on
slot 4+j (fully masked for p=0 / diag for p=1). A/B get per-core
multiplicative fp16 input masks (tri/ones, zeros/tri), keeping the
program identical while the data differs.

fp16 operands throughout (1.0 PE cycles/row, half the DMA bytes).
Softmax denominators: masked exp tiles accumulate on GpSimd into a
per-q-slot SBUF f32 accumulator; one ones-matmul per q-slot reduces
the final 128 partitions. Phase-1 transposes lag one 128-block and
phase-2 AV matmuls lag two score matmuls (software pipelining) so the
PE never stalls on DVE/ACT results.
"""
import numpy as np

B, T, C, HD = 4, 4096, 2048, 128
P = 128
BS = 512
SCALE = float(C) ** -0.5


def build():
    import concourse.bass as bass
    import concourse.mybir as mybir
    import bass_rust
    from concourse.tile import TileContext
    from concourse.masks import make_identity

    f32 = mybir.dt.float32
    f32r = mybir.dt.float32r
    f16 = mybir.dt.float16
    EXP = mybir.ActivationFunctionType.Exp

    nc = bass.Bass(num_devices=8)
    xt = nc.declare_dram_parameter("xt", [C, T // 2], f16, isOutput=False)
    w = nc.declare_dram_parameter("w", [C, 3 * HD], f16, isOutput=False)
    cos2 = nc.declare_dram_parameter("cos2", [T // 2, P], f16, isOutput=False)
    sin2 = nc.declare_dram_parameter("sin2", [T // 2, P], f16, isOutput=False)
    mka = nc.declare_dram_parameter("mka", [P, 4 * BS], f16, isOutput=False)
    mkb = nc.declare_dram_parameter("mkb", [P, 4 * BS], f16, isOutput=False)
    out = nc.declare_dram_parameter("out", [T // 2, HD], f32, isOutput=True)

    cins = [nc.dram_tensor(f"cin{t}", [P, 2 * BS], f16, kind="Internal")
            for t in range(4)]
    couts = [nc.dram_tensor(f"cout{t}", [2 * P, 2 * BS], f16, kind="Internal")
             for t in range(4)]

    with TileContext(nc) as tc:
        with (
            tc.tile_pool(name="const", bufs=1) as cp,
            tc.tile_pool(name="xp", bufs=2) as xp,
            tc.tile_pool(name="rot", bufs=3) as rp,
            tc.tile_pool(name="pt", bufs=4) as ptp,
            tc.tile_pool(name="osb", bufs=2) as osb,
            tc.tile_pool(name="tps", bufs=2, space="PSUM") as tps,
        ):
            # ---- constants / resident tensors ----
            ident = cp.tile([P, P], f16, tag="ident")
            make_identity(nc, ident[:])
            ones = cp.tile([P, 1], f16, tag="ones")
            nc.gpsimd.memset(ones[:], 1.0)
            one11 = cp.tile([1, 1], f32, tag="one11")
            nc.gpsimd.memset(one11[:], 1.0)
            mA = cp.tile([P, 4 * BS], f16, tag="mA")
            nc.sync.dma_start(mA[:], mka[:])
            mB = cp.tile([P, 4 * BS], f16, tag="mB")
            nc.sync.dma_start(mB[:], mkb[:])
            wt = cp.tile([P, 16 * 384], f16, tag="wt")
            for g in range(4):
                nc.sync.dma_start(
                    wt[:, g * 4 * 384:(g + 1) * 4 * 384].rearrange(
                        "p (k n) -> p k n", k=4),
                    w[g * 512:(g + 1) * 512, :].rearrange(
                        "(k p) n -> p k n", p=P))
            cst = cp.tile([P, 16 * P], f16, tag="cst")
            snt = cp.tile([P, 16 * P], f16, tag="snt")
            for g in range(2):
                sl = slice(g * 8 * P, (g + 1) * 8 * P)
                nc.sync.dma_start(
                    cst[:, sl].rearrange("p (k n) -> p k n", k=8),
                    cos2[g * 8 * P:(g + 1) * 8 * P, :].rearrange(
                        "(k p) n -> p k n", p=P))
                nc.sync.dma_start(
                    snt[:, sl].rearrange("p (k n) -> p k n", k=8),
                    sin2[g * 8 * P:(g + 1) * 8 * P, :].rearrange(
                        "(k p) n -> p k n", p=P))

            qT = cp.tile([P, 16 * P], f16, tag="qT")    # [d, 2048] own q
            kTm = cp.tile([P, 16 * P], f16, tag="kTm")  # own kT (slot order)
            vm = cp.tile([P, 16 * P], f16, tag="vm")    # own v s-tiles
            kT = cp.tile([P, 32 * P], f16, tag="kT")    # rank-ordered [d, 4096]
            vsb = cp.tile([P, 32 * P], f16, tag="vsb")  # rank-ordered v s-tiles
            # denominator partial accumulators per q-slot j: even-st tiles
            # accumulate on GpSimd into accA, odd-st on DVE into accB, so
            # neither engine saturates against the PE's st-pass rate
            accA = cp.tile([P, 4 * BS], f16, tag="accA")
            accB = cp.tile([P, 4 * BS], f16, tag="accB")
            for j in range(4):
                nc.gpsimd.memset(accA[:, j * BS:(j + 1) * BS], 0.0)
                nc.vector.memset(accB[:, j * BS:(j + 1) * BS], 0.0)

            # ---- phase 1: projection + RoPE + transpose (own rows) ----
            # transposes lag one t128 so the PE never waits on DVE RoPE
            H = 64
            pending = []

            def flush_pending():
                while pending:
                    src, dstcol = pending.pop(0)
                    tp = tps.tile([P, P], f16, tag="tp")
                    nc.tensor.transpose(tp[:], src[:], ident[:])
                    dst = qT if dstcol[0] == "q" else kTm
                    nc.scalar.copy(dst[:, dstcol[1] * P:(dstcol[1] + 1) * P],
                                   tp[:])

            def issue_exchange(tg):
                nc.sync.dma_start(cins[tg][:, 0:BS],
                                  kTm[:, tg * BS:(tg + 1) * BS])
                nc.sync.dma_start(cins[tg][:, BS:2 * BS],
                                  vm[:, tg * BS:(tg + 1) * BS])
                nc.gpsimd.collective_compute(
                    "AllGather", mybir.AluOpType.bypass,
                    replica_groups=[[0, 1], [2, 3], [4, 5], [6, 7]],
                    ins=[cins[tg][:]], outs=[couts[tg][:]],
                )
                for r in range(2):
                    scol = (4 * r + tg) * BS
                    nc.sync.dma_start(kT[:, scol:scol + BS],
                                      couts[tg][r * P:(r + 1) * P, 0:BS])
                    nc.sync.dma_start(vsb[:, scol:scol + BS],
                                      couts[tg][r * P:(r + 1) * P, BS:2 * BS])

            with tc.tile_pool(name="pps", bufs=2, space="PSUM") as pps:
                for tg in range(4):
                    xts = []
                    for ci in range(16):
                        xtile = xp.tile([P, BS], f16, tag=f"x{ci}")
                        nc.sync.dma_start(
                            xtile[:], xt[ci * P:(ci + 1) * P,
                                         tg * BS:(tg + 1) * BS])
                        xts.append(xtile)
                    for sub in range(4):
                        t128 = tg * 4 + sub
                        pp = pps.tile([P, 384], f32, tag="pp")
                        for ci in range(16):
                            nc.tensor.matmul(
                                pp[:],
                                xts[ci][:, sub * P:(sub + 1) * P],
                                wt[:, ci * 384:(ci + 1) * 384],
                                start=(ci == 0), stop=(ci == 15))
                        # issue the previous chunk's exchange + lagged
                        # transposes while this t128's projection runs
                        flush_pending()
                        if sub == 0 and tg > 0:
                            issue_exchange(tg - 1)
                        cs = cst[:, t128 * P:(t128 + 1) * P]
                        sn = snt[:, t128 * P:(t128 + 1) * P]

                        def rope(src_off, dst):
                            s0 = pp[:, src_off:src_off + P]
                            nc.vector.tensor_mul(dst[:], s0, cs)
                            tmp = rp.tile([P, P], f16, tag="ropetmp")
                            nc.vector.tensor_mul(
                                tmp[:, 0:H], pp[:, src_off + H:src_off + P],
                                sn[:, 0:H])
                            nc.vector.tensor_mul(
                                tmp[:, H:P], pp[:, src_off:src_off + H],
                                sn[:, H:P])
                            nc.vector.tensor_add(dst[:], dst[:], tmp[:])

                        rk = rp.tile([P, P], f16, tag="rk")
                        rope(0, rk)
                        nc.scalar.copy(vm[:, t128 * P:(t128 + 1) * P],
                                       pp[:, P:2 * P])
                        rq = rp.tile([P, P], f16, tag="rq")
                        rope(2 * P, rq)
                        pending.append((rk, ("k", t128)))
                        pending.append((rq, ("q", t128)))
                flush_pending()
                issue_exchange(3)

            # ---- phase 2: attention, q-slot j ascending ----
            # st-pass list with software-pipeline depth 2 on the PE
            with (
                tc.tile_pool(name="sps", bufs=3, space="PSUM") as sps,
                tc.tile_pool(name="o2ps", bufs=2, space="PSUM") as o2ps,
                tc.tile_pool(name="smps", bufs=1, space="PSUM") as smps,
            ):
                o2s = {}

                def finalize(j):
                    o2 = o2s[j]
                    jsl = slice(j * BS, (j + 1) * BS)
                    sm = smps.tile([1, BS], f32, tag="sm")
                    nc.tensor.matmul(sm[:], ones[:], accA[:, jsl],
                                     start=True, stop=False)
                    nc.tensor.matmul(sm[:], ones[:], accB[:, jsl],
                                     start=False, stop=True)
                    smsb = osb.tile([1, BS], f32, tag="smsb")
                    nc.scalar.copy(smsb[:], sm[:])
                    o2sb = osb.tile([P, BS], f16, tag="o2sb")
                    nc.scalar.copy(o2sb[:], o2[:])
                    rcp = osb.tile([P, 4], f32, tag="rcp")
                    for ch in range(4):
                        rs = tps.tile([P, 1], f32, tag="tp")
                        nc.tensor.transpose(rs[:],
                                            smsb[0:1, ch * P:(ch + 1) * P],
                                            one11[:])
                        nc.vector.reciprocal(rcp[:, ch:ch + 1], rs[:])
                    for ch in range(4):
                        ot = tps.tile([P, P], f16, tag="tp")
                        nc.tensor.transpose(ot[:], o2sb[:, ch * P:(ch + 1) * P],
                                            ident[:])
                        osbt = osb.tile([P, P], f32, tag="ofin")
                        nc.vector.tensor_scalar_mul(osbt[:], ot[:],
                                                    rcp[:, ch:ch + 1])
                        r0 = j * BS + ch * P
                        nc.sync.dma_start(out[r0:r0 + P, :], osbt[:])

                # build the flat list of st-passes
                plan = []  # (j, scol, mask_or_None, first, last)
                for j in range(4):
                    passes = ([(s, None) for s in range(j)]
                              + [(4 + s, None) for s in range(j)]
                              + [(j, mA), (4 + j, mB)])
                    npass = len(passes)
                    for pi, (si, mask) in enumerate(passes):
                        for st in range(4):
                            plan.append((j, si * BS + st * P,
                                         None if mask is None
                                         else mask[:, st * BS:(st + 1) * BS],
                                         pi == 0 and st == 0,
                                         pi == npass - 1 and st == 3))

                inflight = []

                def emit_scores(item):
                    j, scol, mask, first, last = item
                    Sps = sps.tile([P, BS], f32, tag="S")
                    nc.tensor.matmul(Sps[:], kT[:, scol:scol + P],
                                     qT[:, j * BS:(j + 1) * BS],
                                     start=True, stop=True)
                    Pt = ptp.tile([P, BS], f16, tag="Pt")
                    nc.scalar.activation(Pt[:], Sps[:], EXP, scale=SCALE)
                    if mask is not None:
                        nc.vector.tensor_mul(Pt[:], Pt[:], mask)
                    return (j, scol, Pt, first, last)

                def emit_av(st8):
                    j, scol, Pt, first, last = st8
                    if first:
                        o2t = o2ps.tile([P, BS], f32, tag="o2")
                        o2s[j] = o2t
                    nc.tensor.matmul(o2s[j][:], vsb[:, scol:scol + P], Pt[:],
                                     start=first, stop=last)
                    jsl = slice(j * BS, (j + 1) * BS)
                    if (scol // P) % 2 == 0:
                        nc.gpsimd.tensor_add(accA[:, jsl], accA[:, jsl], Pt[:])
                    else:
                        nc.vector.tensor_add(accB[:, jsl], accB[:, jsl], Pt[:])
                    if last:
                        finalize(j)

                for item in plan:
                    inflight.append(emit_scores(item))
                    if len(inflight) > 2:
                        emit_av(inflight.pop(0))
                while inflight:
                    emit_av(inflight.pop(0))

    bass_rust.generate_event_semaphores(nc)
    return nc


_CACHE = {}


def _get_nc():
    if "nc" not in _CACHE:
        _CACHE["nc"] = build()
    return _CACHE["nc"]


def _prep_inputs(x, Wq, Wk, Wv, cos, sin):
    perm = np.concatenate([np.arange(0, HD, 2), np.arange(1, HD, 2)])
    wq = Wq[perm].astype(np.float32)
    wk = Wk[perm].astype(np.float32)
    w = np.concatenate([wk.T, Wv.T.astype(np.float32), wq.T],
                       axis=1).astype(np.float16)  # [C, 384] = [k|v|q]
    cos2 = np.concatenate([cos, cos], axis=1).astype(np.float16)
    sin2 = np.concatenate([-sin, sin], axis=1).astype(np.float16)
    s = np.arange(P)[:, None]
    q = np.arange(BS)[None, :]
    tri = np.concatenate(
        [(s + P * st <= q).astype(np.float16) for st in range(4)], axis=1)
    ones_m = np.ones((P, 4 * BS), np.float16)
    zeros_m = np.zeros((P, 4 * BS), np.float16)
    in_maps, orders = [], []
    for c in range(8):
        b, par = c // 2, c % 2
        order = [par, par + 2, par + 4, par + 6]
        orders.append(order)
        xb = np.asarray(x[b], np.float32)
        xtp = np.empty((C, T // 2), np.float16)
        c2 = np.empty((T // 2, P), np.float16)
        s2 = np.empty((T // 2, P), np.float16)
        for sl, ab in enumerate(order):
            dst = slice(sl * BS, (sl + 1) * BS)
            src = slice(ab * BS, (ab + 1) * BS)
            xtp[:, dst] = xb[src].T
            c2[dst] = cos2[src]
            s2[dst] = sin2[src]
        in_maps.append({"xt": np.ascontiguousarray(xtp), "w": w,
                        "cos2": np.ascontiguousarray(c2),
                        "sin2": np.ascontiguousarray(s2),
                        "mka": tri if par == 0 else ones_m,
                        "mkb": zeros_m if par == 0 else tri})
    return in_maps, orders


def _run(x, Wq, Wk, Wv, cos, sin, trace=False):
    from concourse.bass_utils import run_bass_kernel_spmd
    nc = _get_nc()
    in_maps, orders = _prep_inputs(x, Wq, Wk, Wv, cos, sin)
    res = run_bass_kernel_spmd(nc, in_maps, list(range(8)), trace=trace)
    full = np.empty((B, T, HD), np.float32)
    for c in range(8):
        b, order = c // 2, orders[c]
        oc = res.results[c]["out"]
        for j in range(4):
            ab = order[j]
            full[b, ab * BS:(ab + 1) * BS] = oc[j * BS:(j + 1) * BS]
    return full, res


def kernel(x, Wq, Wk, Wv, cos, sin):
    return _run(x, Wq, Wk, Wv, cos, sin, trace=False)[0]
